# revision 1
# baseline (speedup 1.0000x reference)
"""Local (sliding-window causal) attention kernel for Trainium2, 8 NeuronCores.

Reference computation (per batch b, head h):
  q = x @ Wq + bq ; k = x @ Wk + bk ; v = x @ Wv + bv   (split into 16 heads of 64)
  S = q k^T / 8, masked to the causal band  i-255 <= j <= i
  out = softmax(S) @ v

Sharding: B=2, H=16 -> 32 (b,h) units; each of 8 cores owns 2 heads x 2 batches
(= a 128-wide column slice of the QKV projections and of the output). Inputs are
replicated (hidden_states as a pre-transposed bf16 x^T) and weights are column-
sliced per core, so no collectives are needed.

Device-side scheme per core (all matmuls in bf16, accumulating in fp32 PSUM):
  1. Q^T, K^T = W^T @ x^T   -> [128 (2 heads*64), 4096] layout (dh on partitions)
  2. V       = x @ Wv       -> [tokens, 128] layout (tokens on partitions),
               stored per 128-token block with a ones-column appended: V' = [V | 1]
  3. Per (b, h, key-block kb of 128 keys): the only queries attending these keys
     are the 384 starting at kb*128, so one matmul
        S^T[kb] = K^T[kb-block].T @ Q^T[:, window]   ([128 keys, <=384 queries])
     + additive band mask + exp (no max-subtraction needed: |scores| < ~4)
     gives P~^T. Then for each 128-query block qb in the window:
        O~[qb] (+)= P~^T[:, qb].T @ V'[kb]           ([128 q, 65]; col 64 = row sums)
     accumulated in PSUM over the <=3 contributing key blocks; finally
     out[qb] = O~[:, :64] * (1 / O~[:, 64]).
bv is folded in on the host: softmax rows sum to 1, so P @ (1 bv^T) = bv.
"""

import os
import sys

import numpy as np

try:
    import concourse.bass as bass  # noqa: F401
except ImportError:
    sys.path.insert(0, "/opt/trn_rl_repo")

import concourse.bass as bass
import concourse.tile as tile
from concourse import bacc, mybir
from concourse.bass import ts
from concourse.bass_utils import run_bass_kernel_spmd

import ml_dtypes

P = 128
B, L, D = 2, 2048, 1024
NT = B * L            # 4096 tokens
KSUB = D // P         # 8 contraction subtiles
CHUNK = 512           # projection chunk (tokens)
NCH = NT // CHUNK     # 8
NLB = NT // P         # 32 token blocks
NKB = L // P          # 16 key blocks per batch
QW = 384              # query window per key block
DH = 64               # head dim
NCORES = 8
HEADS_PER_CORE = 2

F32 = mybir.dt.float32
BF16 = mybir.dt.bfloat16

VARIANT = "full"  # bisect hook: full | proj | projv | noatt... (see build_program)


def build_program():
    nc = bacc.Bacc("TRN2", target_bir_lowering=False, debug=False,
                   num_devices=NCORES)

    xt_d = nc.dram_tensor("xt", [P, KSUB, NT], BF16, kind="ExternalInput").ap()
    wq_d = nc.dram_tensor("wq", [P, KSUB, P], BF16, kind="ExternalInput").ap()
    wk_d = nc.dram_tensor("wk", [P, KSUB, P], BF16, kind="ExternalInput").ap()
    wv_d = nc.dram_tensor("wv", [P, KSUB, P], BF16, kind="ExternalInput").ap()
    bq_d = nc.dram_tensor("bq", [P, 1], F32, kind="ExternalInput").ap()
    bk_d = nc.dram_tensor("bk", [P, 1], F32, kind="ExternalInput").ap()
    mask_d = nc.dram_tensor("mask", [P, QW], F32, kind="ExternalInput").ap()
    out_d = nc.dram_tensor("out", [B, L, P], F32, kind="ExternalOutput").ap()

    with tile.TileContext(nc) as tc:
        with (
            tc.tile_pool(name="const", bufs=1) as const,
            tc.tile_pool(name="xtp", bufs=1) as xtp,
            tc.tile_pool(name="qkv", bufs=1) as qkv,
        ):
            mask_sb = const.tile([P, QW], F32)
            nc.sync.dma_start(mask_sb[:], mask_d)
            wq_sb = const.tile([P, KSUB, P], BF16)
            nc.sync.dma_start(wq_sb[:], wq_d)
            wk_sb = const.tile([P, KSUB, P], BF16)
            nc.sync.dma_start(wk_sb[:], wk_d)
            wv_sb = const.tile([P, KSUB, P], BF16)
            nc.sync.dma_start(wv_sb[:], wv_d)
            bq_sb = const.tile([P, 1], F32)
            nc.sync.dma_start(bq_sb[:], bq_d)
            bk_sb = const.tile([P, 1], F32)
            nc.sync.dma_start(bk_sb[:], bk_d)

            qt_sb = qkv.tile([P, NT], BF16, tag="qt")   # Q^T (2 heads on partitions)
            kt_sb = qkv.tile([P, NT], BF16, tag="kt")   # K^T
            v_sb = qkv.tile([P, HEADS_PER_CORE, NLB, DH + 1], BF16, tag="v")
            nc.vector.memset(v_sb[:, :, :, DH:DH + 1], 1.0)

            xts = []
            for c in range(NCH):
                t = xtp.tile([P, KSUB, CHUNK], BF16, tag=f"xt{c}")
                nc.sync.dma_start(t[:], xt_d[:, :, ts(c, CHUNK)])
                xts.append(t)

            do_p1 = VARIANT in ("full", "p1", "p12", "p13")
            do_p2 = VARIANT in ("full", "p12", "p2")
            do_p3 = VARIANT in ("full", "p13")
            if not do_p3:
                dummy = qkv.tile([P, DH], F32, tag="dummy")
                nc.vector.memset(dummy[:], 0.0)
                for b in range(B):
                    for qb in range(NKB):
                        for h in range(HEADS_PER_CORE):
                            nc.sync.dma_start(
                                out_d[b, qb * P:(qb + 1) * P,
                                      h * DH:(h + 1) * DH], dummy[:])

            # ---- Fused per-batch pipeline: projections + attention ----
            # Attention key-blocks issue as soon as their 384-token QT/KT
            # window and V' blocks exist, so ACT/DVE softmax work overlaps
            # the projection matmuls instead of running after them.
            with (
                tc.tile_pool(name="pjps", bufs=2, space="PSUM") as pj_ps,
                tc.tile_pool(name="vps", bufs=1, space="PSUM") as v_ps,
                tc.tile_pool(name="stps", bufs=2, space="PSUM") as st_ps,
                tc.tile_pool(name="ops", bufs=3, space="PSUM") as o_ps,
                tc.tile_pool(name="att", bufs=6) as att,
                tc.tile_pool(name="ptp", bufs=8) as ptp,
                tc.tile_pool(name="osb", bufs=6) as osb,
            ):
                def attend(b, kb, o_tiles, o_outs):
                    t0 = b * L
                    k0 = t0 + kb * P
                    qw = min(QW, L - kb * P)
                    for h in range(HEADS_PER_CORE):
                        hs = h * DH
                        ps_st = st_ps.tile([P, QW], F32, tag="st", name="ps_st")
                        nc.tensor.matmul(ps_st[:, :qw],
                                         lhsT=kt_sb[hs:hs + DH, k0:k0 + P],
                                         rhs=qt_sb[hs:hs + DH, k0:k0 + qw],
                                         start=True, stop=True)
                        st_sb = att.tile([P, QW], F32, tag="st_sb",
                                         name="st_sb")
                        nc.vector.tensor_add(st_sb[:, :qw], ps_st[:, :qw],
                                             mask_sb[:, :qw])
                        pt_sb = ptp.tile([P, QW], BF16, tag="pt", name="pt_sb")
                        nc.scalar.activation(
                            pt_sb[:, :qw], st_sb[:, :qw],
                            mybir.ActivationFunctionType.Exp, scale=0.125)
                        for qb in range(kb, min(kb + 3, NKB)):
                            qoff = (qb - kb) * P
                            first = (kb == max(qb - 2, 0))
                            last = (qb == kb)
                            if first and h == 0:
                                o_tiles[qb] = o_ps.tile(
                                    [P, 2 * (DH + 1)], F32, tag="o",
                                    name=f"o_{b}_{qb}")
                            osl = o_tiles[qb][:, h * (DH + 1):
                                              (h + 1) * (DH + 1)]
                            # start=True clears has_written for the WHOLE
                            # bank, so only h0 may issue it; h1's first
                            # matmul lands on freshly cleared bits and
                            # overwrites, later ones accumulate.
                            nc.tensor.matmul(
                                osl,
                                lhsT=pt_sb[:, qoff:qoff + P],
                                rhs=v_sb[:, h, b * NKB + kb, :],
                                start=first and h == 0, stop=last,
                                skip_group_check=True)
                            if last:
                                ot = o_tiles[qb]
                                if h == 1:
                                    o_tiles.pop(qb)
                                c0 = h * (DH + 1)
                                r = osb.tile([P, 1], F32, tag="r", name="r")
                                nc.vector.reciprocal(
                                    r[:], ot[:, c0 + DH:c0 + DH + 1])
                                if h == 0:
                                    o_outs[qb] = osb.tile(
                                        [P, 2 * DH], F32, tag="oo",
                                        name=f"oo_{b}_{qb}")
                                o_out = o_outs[qb]
                                nc.vector.tensor_scalar_mul(
                                    o_out[:, hs:hs + DH],
                                    ot[:, c0:c0 + DH], r[:])
                                if h == 1:
                                    nc.sync.dma_start(
                                        out_d[b, qb * P:(qb + 1) * P, :],
                                        o_outs.pop(qb)[:])

                # kbs whose QT/KT window completes with local chunk cc
                ready = {0: [0, 1], 1: [2, 3, 4, 5], 2: [6, 7, 8, 9],
                         3: [10, 11, 12, 13]}
                for b in range(B if (do_p1 and do_p2 and do_p3) else 0):
                    o_tiles, o_outs = {}, {}
                    for cc in range(4):
                        c = b * 4 + cc
                        for w_sb, b_sb, dst in ((wq_sb, bq_sb, qt_sb),
                                                (wk_sb, bk_sb, kt_sb)):
                            ps = pj_ps.tile([P, CHUNK], F32, tag="pj",
                                            name="pj")
                            for k in range(KSUB):
                                nc.tensor.matmul(ps[:], lhsT=w_sb[:, k, :],
                                                 rhs=xts[c][:, k, :],
                                                 start=(k == 0),
                                                 stop=(k == KSUB - 1))
                            nc.vector.tensor_scalar_add(dst[:, ts(c, CHUNK)],
                                                        ps[:], b_sb[:, 0:1])
                        for lo in range(4):
                            lb = c * 4 + lo
                            ps = v_ps.tile([P, P], F32, tag="v", name="vps")
                            for k in range(KSUB):
                                nc.tensor.matmul(
                                    ps[:], lhsT=xts[c][:, k, ts(lo, P)],
                                    rhs=wv_sb[:, k, :],
                                    start=(k == 0), stop=(k == KSUB - 1))
                            for h in range(HEADS_PER_CORE):
                                nc.vector.tensor_copy(
                                    v_sb[:, h, lb, 0:DH],
                                    ps[:, h * DH:(h + 1) * DH])
                        for kb in ready[cc]:
                            attend(b, kb, o_tiles, o_outs)
                    for kb in (14, 15):
                        attend(b, kb, o_tiles, o_outs)
    nc.finalize()
    return nc


_NC = None


def _get_nc():
    global _NC
    if _NC is None:
        _NC = build_program()
    return _NC


def _band_mask():
    pk = np.arange(P)[:, None]
    fq = np.arange(QW)[None, :]
    valid = (fq >= pk) & (fq - pk <= 255)
    return np.where(valid, 0.0, -30000.0).astype(np.float32)


def _prepare_in_maps(inputs):
    hs = np.asarray(inputs["hidden_states"], np.float32)
    Wq = np.asarray(inputs["Wq"], np.float32)
    Wk = np.asarray(inputs["Wk"], np.float32)
    Wv = np.asarray(inputs["Wv"], np.float32)
    bq = np.asarray(inputs["bq"], np.float32)
    bk = np.asarray(inputs["bk"], np.float32)

    x_flat = hs.reshape(NT, D)
    # xt[p, k, t] = x_flat[t, k*128+p]
    xt = np.ascontiguousarray(
        x_flat.T.reshape(KSUB, P, NT).transpose(1, 0, 2)
    ).astype(ml_dtypes.bfloat16)
    mask = _band_mask()

    def wslice(W, c):
        # [P, KSUB, 128]: w[p, k, m] = W[k*128+p, c*128+m]
        return np.ascontiguousarray(
            W[:, c * P:(c + 1) * P].reshape(KSUB, P, P).transpose(1, 0, 2)
        ).astype(ml_dtypes.bfloat16)

    in_maps = []
    for c in range(NCORES):
        in_maps.append({
            "xt": xt,
            "wq": wslice(Wq, c),
            "wk": wslice(Wk, c),
            "wv": wslice(Wv, c),
            "bq": np.ascontiguousarray(bq[c * P:(c + 1) * P].reshape(P, 1)),
            "bk": np.ascontiguousarray(bk[c * P:(c + 1) * P].reshape(P, 1)),
            "mask": mask,
        })
    return in_maps


def run(inputs, trace=False, **kwargs):
    nc = _get_nc()
    in_maps = _prepare_in_maps(inputs)
    res = run_bass_kernel_spmd(nc, in_maps, core_ids=list(range(NCORES)),
                               trace=trace, **kwargs)
    bv = np.asarray(inputs["bv"], np.float32)
    full = np.concatenate([res.results[c]["out"] for c in range(NCORES)],
                          axis=2)
    full = full + bv[None, None, :]
    return full.astype(np.float32), res


def kernel(**inputs):
    out, _ = run(inputs, trace=False)
    return out



# revision 5
# speedup vs baseline: 1.3102x; 1.3102x over previous
"""Local (sliding-window causal) attention kernel for Trainium2, 8 NeuronCores.

Reference computation (per batch b, head h):
  q = x @ Wq + bq ; k = x @ Wk + bk ; v = x @ Wv + bv   (16 heads of 64)
  S = q k^T / 8, masked to the causal band  i-255 <= j <= i
  out = softmax(S) @ v

Sharding: B=2, H=16 -> each of 8 cores owns a 128-wide column slice of the
QKV projections (2 heads) for both batches. Inputs are replicated; weights
column-sliced per core; no collectives.

v2 scheme (all matmuls fp8 DoubleRow for projections, bf16 for attention):
  - x is shipped as an fp8 pair (xh = fp8(x^T), xl = fp8(x^T - xh)); weights
    as fp8 pairs of 64*W (64x scaling keeps W ~N(0,0.02) in e4m3 normal
    range). Projections use 3 correction terms:
       64*q = xh@wq8 + xh@wql + xl@wq8      (same for k, v)
    accumulated in PSUM via DoubleRow (2 k-subtiles per pass), then a
    tensor_scalar copy rescales by 1/64 (+bias) into bf16 SBUF.
  - Attention per (b, key-block kb of 128): S^T for both heads lands in one
    2-bank PSUM tile; one ACT exp (scale=1/8) -> P~^T bf16; DVE multiplies
    the two triangular 0/1 masks (diag cols 0:128, tail cols 256:384; the
    middle 128 are always in-band). PV matmuls accumulate [128q, 65] per
    (qb, h) into per-3-qb PSUM "super" tiles (col 64 = row sums via the
    ones-column of V'); a DVE copy stages [128, 3, 130] to SBUF and the
    result ships unnormalized; the host divides by the row sums and adds bv.
"""

import sys

import numpy as np

try:
    import concourse.bass as bass  # noqa: F401
except ImportError:
    sys.path.insert(0, "/opt/trn_rl_repo")

import concourse.bass as bass  # noqa: F401
import concourse.tile as tile
from concourse import bacc, mybir
from concourse.bass import ts
from concourse.bass_utils import run_bass_kernel_spmd

import ml_dtypes

P = 128
B, L, D = 2, 2048, 1024
NT = B * L            # 4096 tokens
KSUB = D // P         # 8 contraction subtiles (4 DoubleRow pairs)
CHUNK = 512           # projection chunk (tokens)
NCH = NT // CHUNK     # 8
G = 256               # DoubleRow token group (rhs free = 2*G = 512)
NLB = NT // P         # 32 token blocks
NKB = L // P          # 16 key blocks per batch
QW = 384              # query window per key block
DH = 64               # head dim
NCORES = 8
WS = 64.0             # weight pre-scale for fp8

F32 = mybir.dt.float32
BF16 = mybir.dt.bfloat16
FP8 = mybir.dt.float8e4

DR = mybir.MatmulPerfMode.DoubleRow

VARIANT = "full"


def build_program():
    nc = bacc.Bacc("TRN2", target_bir_lowering=False, debug=False,
                   num_devices=NCORES)

    xh_d = nc.dram_tensor("xh", [P, KSUB, NT], FP8, kind="ExternalInput").ap()
    xl_d = nc.dram_tensor("xl", [P, KSUB, NT], FP8, kind="ExternalInput").ap()
    w_ds = {}
    for wn in ("wq8", "wql", "wk8", "wkl", "wvh", "wvl"):
        w_ds[wn] = nc.dram_tensor(wn, [P, KSUB, P], FP8,
                                  kind="ExternalInput").ap()
    bq_d = nc.dram_tensor("bq", [P, 1], F32, kind="ExternalInput").ap()
    bk_d = nc.dram_tensor("bk", [P, 1], F32, kind="ExternalInput").ap()
    mkd_d = nc.dram_tensor("mkd", [P, 2, P], BF16, kind="ExternalInput").ap()
    mkt_d = nc.dram_tensor("mkt", [P, 2, P], BF16, kind="ExternalInput").ap()
    out_d = nc.dram_tensor("out", [B, L, 2 * (DH + 1)], F32,
                           kind="ExternalOutput").ap()

    with tile.TileContext(nc) as tc:
        with (
            tc.tile_pool(name="const", bufs=1) as const,
            tc.tile_pool(name="qkv", bufs=1) as qkv,
            tc.tile_pool(name="xhp", bufs=3) as xhp,
            tc.tile_pool(name="xlp", bufs=3) as xlp,
            tc.tile_pool(name="ptp", bufs=5) as ptp,
            tc.tile_pool(name="ostp", bufs=3) as ostp,
            tc.tile_pool(name="pjps", bufs=2, space="PSUM") as pj_ps,
            tc.tile_pool(name="stps", bufs=2, space="PSUM") as st_ps,
            tc.tile_pool(name="ops", bufs=2, space="PSUM") as o_ps,
        ):
            w_sb = {}
            for wn, wd in w_ds.items():
                w_sb[wn] = const.tile([P, KSUB, P], FP8, tag=wn, name=wn)
                nc.sync.dma_start(w_sb[wn][:], wd)
            mkd_sb = const.tile([P, 2, P], BF16, tag="mkd")
            nc.sync.dma_start(mkd_sb[:], mkd_d)
            mkt_sb = const.tile([P, 2, P], BF16, tag="mkt")
            nc.sync.dma_start(mkt_sb[:], mkt_d)
            bq_sb = const.tile([P, 1], F32, tag="bq")
            nc.sync.dma_start(bq_sb[:], bq_d)
            bk_sb = const.tile([P, 1], F32, tag="bk")
            nc.sync.dma_start(bk_sb[:], bk_d)

            qt_sb = qkv.tile([P, NT], BF16, tag="qt")   # 2 heads' dh on parts
            kt_sb = qkv.tile([P, NT], BF16, tag="kt")
            v_sb = qkv.tile([P, 2, NLB, DH + 1], BF16, tag="v")
            nc.vector.memset(v_sb[:, :, :, DH:DH + 1], 1.0)

            xhs, xls = [], []
            for c in range(NCH):
                th = xhp.tile([P, KSUB, CHUNK], FP8, tag=f"xh{c % 3}",
                              name=f"xh{c}")
                nc.sync.dma_start(th[:], xh_d[:, :, ts(c, CHUNK)])
                xhs.append(th)
                tl = xlp.tile([P, KSUB, CHUNK], FP8, tag=f"xl{c % 3}",
                              name=f"xl{c}")
                nc.sync.dma_start(tl[:], xl_d[:, :, ts(c, CHUNK)])
                xls.append(tl)

            def proj_qk(c, half, w8, wl, bias, dst):
                """One 256-token DoubleRow group for Q^T or K^T."""
                g0 = half * G
                pj = pj_ps.tile([P, 2 * G], F32, tag="pj", name="pj")
                sl = pj[:, 0:G] if half == 0 else pj[:, G:2 * G]
                # reuse one [P,512] bank for both halves to halve bank count
                terms = ((w8, xhs[c]), (wl, xhs[c]), (w8, xls[c]))
                n = 0
                for wt, xt in terms:
                    for kp in range(KSUB // 2):
                        nc.tensor.matmul(
                            sl, lhsT=wt[:, 2 * kp:2 * kp + 2, :],
                            rhs=xt[:, 2 * kp:2 * kp + 2, g0:g0 + G],
                            start=(n == 0), stop=(n == 11),
                            perf_mode=DR, skip_group_check=True)
                        n += 1
                nc.vector.tensor_scalar(
                    dst[:, c * CHUNK + g0:c * CHUNK + g0 + G], sl,
                    1.0 / WS, bias[:, 0:1],
                    op0=mybir.AluOpType.mult, op1=mybir.AluOpType.add)

            def proj_v(c):
                """V for one 512-token chunk: 4 lb blocks in one PSUM bank."""
                pv = pj_ps.tile([P, 4, P], F32, tag="pj", name="pv")
                terms = ((w_sb["wvh"], xhs[c]), (w_sb["wvl"], xhs[c]),
                         (w_sb["wvh"], xls[c]))
                for lo in range(4):
                    n = 0
                    for wt, xt in terms:
                        for kp in range(KSUB // 2):
                            nc.tensor.matmul(
                                pv[:, lo, :],
                                lhsT=xt[:, 2 * kp:2 * kp + 2, ts(lo, P)],
                                rhs=wt[:, 2 * kp:2 * kp + 2, :],
                                start=(n == 0), stop=(n == 11),
                                perf_mode=DR, skip_group_check=True)
                            n += 1
                for h in range(2):
                    nc.vector.tensor_scalar_mul(
                        v_sb[:, h, 4 * c:4 * c + 4, 0:DH],
                        pv[:, :, h * DH:(h + 1) * DH], 1.0 / WS)

            o_tiles = {}
            o_done = {}

            def flush_super(b, s):
                """Copy a finished 3-qb PSUM super tile to SBUF + DMA out."""
                nslots = min(3, NKB - 3 * s)
                ot = o_tiles.pop((b, s))
                st = ostp.tile([P, 3, 2 * (DH + 1)], F32, tag="ost",
                               name=f"ost_{b}_{s}")
                nc.vector.tensor_copy(st[:, 0:nslots, :], ot[:, 0:nslots, :])
                q0 = 3 * s * P
                nc.scalar.dma_start(
                    out_d[b, q0:q0 + nslots * P, :]
                    .rearrange("(s p) c -> p s c", p=P),
                    st[:, 0:nslots, :])

            def attend(b, kb, mask_engine):
                t0 = b * L
                k0 = t0 + kb * P
                qw = min(QW, L - kb * P)
                st = st_ps.tile([P, 2, 512], F32, tag="st", name="st")
                for h in range(2):
                    hs = h * DH
                    nc.tensor.matmul(st[:, h, 0:qw],
                                     lhsT=kt_sb[hs:hs + DH, k0:k0 + P],
                                     rhs=qt_sb[hs:hs + DH, k0:k0 + qw],
                                     start=True, stop=True)
                pt = ptp.tile([P, 2, QW], BF16, tag="pt", name="pt")
                nc.scalar.activation(
                    pt[:, :, 0:qw], st[:, :, 0:qw],
                    mybir.ActivationFunctionType.Exp, scale=0.125)
                eng = nc.gpsimd if mask_engine == "pool" else nc.vector
                eng.tensor_mul(pt[:, :, 0:P], pt[:, :, 0:P], mkd_sb[:])
                if qw == QW:
                    eng.tensor_mul(pt[:, :, 2 * P:3 * P],
                                   pt[:, :, 2 * P:3 * P], mkt_sb[:])
                for h in range(2):
                    for qb in range(kb, min(kb + 3, NKB)):
                        s, slot = divmod(qb, 3)
                        qoff = (qb - kb) * P
                        first = (kb == max(qb - 2, 0))
                        if first and slot == 0 and h == 0:
                            o_tiles[(b, s)] = o_ps.tile(
                                [P, 3, 2 * (DH + 1)], F32, tag="o",
                                name=f"o_{b}_{s}")
                        ot = o_tiles[(b, s)]
                        nc.tensor.matmul(
                            ot[:, slot, h * (DH + 1):(h + 1) * (DH + 1)],
                            lhsT=pt[:, h, qoff:qoff + P],
                            rhs=v_sb[:, h, b * NKB + kb, :],
                            start=(first and slot == 0 and h == 0),
                            stop=(qb == kb), skip_group_check=True)
                        if qb == kb and h == 1:
                            done = o_done.get((b, s), 0) + 1
                            o_done[(b, s)] = done
                            nslots = min(3, NKB - 3 * s)
                            if done == nslots:
                                flush_super(b, s)

            # Attend(b, kb) is ready once Q^T/K^T cover token (kb+3)*128 of
            # batch b (i.e. after 256-token group ceil((kb+3)/2) of that
            # batch) and V covers key block kb (after proj_v of chunk
            # kb//4).  Emit each attend at the earliest such point so PE
            # always has matmul work while ACT runs exp.
            nmask_pool = [0]

            def pop_ready(b, pend, groups_done, v_chunks_done):
                while pend:
                    kb = pend[0]
                    if (min(kb + 3, NKB) * P > groups_done * G
                            or kb // 4 >= v_chunks_done):
                        break
                    pend.pop(0)
                    eng = "pool" if nmask_pool[0] < 8 else "dve"
                    if eng == "pool":
                        nmask_pool[0] += 1
                    attend(b, kb, eng)

            for b in range(B):
                pend = list(range(NKB))
                gd, vd = 0, 0
                for cc in range(4):
                    c = b * 4 + cc
                    proj_qk(c, 0, w_sb["wq8"], w_sb["wql"], bq_sb, qt_sb)
                    proj_qk(c, 0, w_sb["wk8"], w_sb["wkl"], bk_sb, kt_sb)
                    gd = 2 * cc + 1
                    pop_ready(b, pend, gd, vd)
                    proj_qk(c, 1, w_sb["wq8"], w_sb["wql"], bq_sb, qt_sb)
                    proj_qk(c, 1, w_sb["wk8"], w_sb["wkl"], bk_sb, kt_sb)
                    gd = 2 * cc + 2
                    pop_ready(b, pend, gd, vd)
                    proj_v(c)
                    vd = cc + 1
                    pop_ready(b, pend, gd, vd)
                pop_ready(b, pend, 100, 100)
                assert not pend
    nc.finalize()
    return nc


_NC = None


def _get_nc():
    global _NC
    if _NC is None:
        _NC = build_program()
    return _NC


def _masks():
    pk = np.arange(P)[:, None]
    f = np.arange(P)[None, :]
    mkd = (f >= pk).astype(np.float32)       # diag block: query >= key
    mkt = (f < pk).astype(np.float32)        # tail block: dist <= 255
    mkd2 = np.repeat(mkd[:, None, :], 2, axis=1).astype(ml_dtypes.bfloat16)
    mkt2 = np.repeat(mkt[:, None, :], 2, axis=1).astype(ml_dtypes.bfloat16)
    return np.ascontiguousarray(mkd2), np.ascontiguousarray(mkt2)


def _prepare_in_maps(inputs):
    hs = np.asarray(inputs["hidden_states"], np.float32)
    Wq = np.asarray(inputs["Wq"], np.float32)
    Wk = np.asarray(inputs["Wk"], np.float32)
    Wv = np.asarray(inputs["Wv"], np.float32)
    bq = np.asarray(inputs["bq"], np.float32)
    bk = np.asarray(inputs["bk"], np.float32)

    x_flat = hs.reshape(NT, D)
    # xt[p, k, t] = x_flat[t, k*128+p]
    xt = np.ascontiguousarray(
        x_flat.T.reshape(KSUB, P, NT).transpose(1, 0, 2))
    xh = xt.astype(ml_dtypes.float8_e4m3)
    xl = (xt - xh.astype(np.float32)).astype(ml_dtypes.float8_e4m3)
    mkd, mkt = _masks()

    def wsplit(W, c):
        # [P, KSUB, 128]: w[p, k, m] = WS * W[k*128+p, c*128+m]
        ws = np.ascontiguousarray(
            (WS * W[:, c * P:(c + 1) * P]).reshape(KSUB, P, P)
            .transpose(1, 0, 2))
        w8 = ws.astype(ml_dtypes.float8_e4m3)
        wl = (ws - w8.astype(np.float32)).astype(ml_dtypes.float8_e4m3)
        return w8, wl

    in_maps = []
    for c in range(NCORES):
        wq8, wql = wsplit(Wq, c)
        wk8, wkl = wsplit(Wk, c)
        wvh, wvl = wsplit(Wv, c)
        in_maps.append({
            "xh": xh, "xl": xl,
            "wq8": wq8, "wql": wql, "wk8": wk8, "wkl": wkl,
            "wvh": wvh, "wvl": wvl,
            "bq": np.ascontiguousarray(bq[c * P:(c + 1) * P].reshape(P, 1)),
            "bk": np.ascontiguousarray(bk[c * P:(c + 1) * P].reshape(P, 1)),
            "mkd": mkd, "mkt": mkt,
        })
    return in_maps


def run(inputs, trace=False, **kwargs):
    nc = _get_nc()
    in_maps = _prepare_in_maps(inputs)
    res = run_bass_kernel_spmd(nc, in_maps, core_ids=list(range(NCORES)),
                               trace=trace, **kwargs)
    bv = np.asarray(inputs["bv"], np.float32)
    outs = []
    for c in range(NCORES):
        o = res.results[c]["out"]  # [B, L, 130]
        for h in range(2):
            outs.append(o[:, :, h * (DH + 1):h * (DH + 1) + DH]
                        / o[:, :, h * (DH + 1) + DH:h * (DH + 1) + DH + 1])
    full = np.concatenate(outs, axis=2)
    full = full + bv[None, None, :]
    return full.astype(np.float32), res


def kernel(**inputs):
    out, _ = run(inputs, trace=False)
    return out


# revision 9
# speedup vs baseline: 1.3505x; 1.0308x over previous
"""Local (sliding-window causal) attention kernel for Trainium2, 8 NeuronCores.

Reference computation (per batch b, head h):
  q = x @ Wq + bq ; k = x @ Wk + bk ; v = x @ Wv + bv   (16 heads of 64)
  S = q k^T / 8, masked to the causal band  i-255 <= j <= i
  out = softmax(S) @ v

Sharding: B=2, H=16 -> each of 8 cores owns a 128-wide column slice of the
QKV projections (2 heads) for both batches. Inputs are replicated; weights
column-sliced per core; no collectives.

Scheme (fp8 DoubleRow projections, bf16 attention):
  - x ships as an fp8 pair (xh = fp8(x^T), xl = fp8(x^T - xh)) in per-chunk
    tensors (contiguous rows -> 1 DMA descriptor per partition); weights as
    fp8 pairs of 64*W (64x scaling keeps W ~N(0,0.02) in e4m3 normal range).
    Projections accumulate correction terms in PSUM via DoubleRow
    (2 k-subtiles per pass):
       64*q = xh@wq8 + xh@wql [+ xl@wq8]     (same for k; v always 3 terms)
    then a tensor_scalar copy rescales by 1/64 (+bias) into bf16 SBUF.
  - Attention per (b, key-block kb of 128): S^T for both heads lands in one
    2-bank PSUM tile; one ACT exp (scale=1/8) -> P~^T bf16; the two
    triangular 0/1 masks multiply in (diag cols 0:128, tail cols 256:384;
    the middle 128 are always in-band) on DVE or Pool. PV matmuls
    accumulate [128q, 65] per (qb, h) into per-3-qb PSUM "super" tiles
    (col 64 = row sums via the ones-column of V'); a DVE copy stages
    [128, 3*130] bf16 to SBUF, shipped unnormalized; the host divides by
    the row sums and adds bv.
"""

import sys

import numpy as np

try:
    import concourse.bass as bass  # noqa: F401
except ImportError:
    sys.path.insert(0, "/opt/trn_rl_repo")

import concourse.bass as bass  # noqa: F401
import concourse.tile as tile
from concourse import bacc, mybir
from concourse.bass_utils import run_bass_kernel_spmd

import ml_dtypes

P = 128
B, L, D = 2, 2048, 1024
NT = B * L            # 4096 tokens
KSUB = D // P         # 8 contraction subtiles (4 DoubleRow pairs)
G = 256               # DoubleRow token group (rhs free = 2*G = 512)
NLB = NT // P         # 32 token blocks
NKB = L // P          # 16 key blocks per batch
QW = 384              # query window per key block
DH = 64               # head dim
OC = 2 * (DH + 1)     # output cols per token (2 heads x (o, rowsum))
NSUP = 6              # supers per batch (3 query blocks each)
NCORES = 8
WS = 64.0             # weight pre-scale for fp8
QK_TERMS = 3          # 3 = full correction, 2 = drop xl@w8 (faster, riskier)

# (start, size) of the x chunks; first two are small to cut startup latency
CHUNKS = [(0, 256), (256, 256), (512, 512), (1024, 512), (1536, 512),
          (2048, 512), (2560, 512), (3072, 512), (3584, 512)]

F32 = mybir.dt.float32
BF16 = mybir.dt.bfloat16
FP8 = mybir.dt.float8e4

DR = mybir.MatmulPerfMode.DoubleRow


def build_program():
    nc = bacc.Bacc("TRN2", target_bir_lowering=False, debug=False,
                   num_devices=NCORES)

    xh_ds, xl_ds = [], []
    for i, (t0, sz) in enumerate(CHUNKS):
        xh_ds.append(nc.dram_tensor(f"xh{i}", [P, KSUB, sz], FP8,
                                    kind="ExternalInput").ap())
        xl_ds.append(nc.dram_tensor(f"xl{i}", [P, KSUB, sz], FP8,
                                    kind="ExternalInput").ap())
    w_ds = {}
    for wn in ("wq8", "wql", "wk8", "wkl", "wvh", "wvl"):
        w_ds[wn] = nc.dram_tensor(wn, [P, KSUB, P], FP8,
                                  kind="ExternalInput").ap()
    bq_d = nc.dram_tensor("bq", [P, 1], F32, kind="ExternalInput").ap()
    bk_d = nc.dram_tensor("bk", [P, 1], F32, kind="ExternalInput").ap()
    mkd_d = nc.dram_tensor("mkd", [P, 2, P], BF16, kind="ExternalInput").ap()
    mkt_d = nc.dram_tensor("mkt", [P, 2, P], BF16, kind="ExternalInput").ap()
    out_d = nc.dram_tensor("out", [B, NSUP, P, 3 * OC], BF16,
                           kind="ExternalOutput").ap()

    with tile.TileContext(nc) as tc:
        with (
            tc.tile_pool(name="const", bufs=1) as const,
            tc.tile_pool(name="qkv", bufs=1) as qkv,
            tc.tile_pool(name="xhp", bufs=4) as xhp,
            tc.tile_pool(name="xlp", bufs=4) as xlp,
            tc.tile_pool(name="ptp", bufs=6) as ptp,
            tc.tile_pool(name="ostp", bufs=3) as ostp,
            tc.tile_pool(name="pjps", bufs=2, space="PSUM") as pj_ps,
            tc.tile_pool(name="stps", bufs=2, space="PSUM") as st_ps,
            tc.tile_pool(name="ops", bufs=2, space="PSUM") as o_ps,
        ):
            w_sb, dmas = {}, {}
            for wn in w_ds:
                w_sb[wn] = const.tile([P, KSUB, P], FP8, tag=wn, name=wn)
            mkd_sb = const.tile([P, 2, P], BF16, tag="mkd")
            mkt_sb = const.tile([P, 2, P], BF16, tag="mkt")
            bq_sb = const.tile([P, 1], F32, tag="bq")
            bk_sb = const.tile([P, 1], F32, tag="bk")

            qt_sb = qkv.tile([P, NT], BF16, tag="qt")   # 2 heads' dh on parts
            kt_sb = qkv.tile([P, NT], BF16, tag="kt")
            v_sb = qkv.tile([P, 2, NLB, DH + 1], BF16, tag="v")
            nc.vector.memset(v_sb[:, :, :, DH:DH + 1], 1.0)

            xhs, xls = [], []
            for i in range(len(CHUNKS)):
                sz = CHUNKS[i][1]
                th = xhp.tile([P, KSUB, sz], FP8, tag=f"xh{i % 4}",
                              name=f"xh{i}")
                tl = xlp.tile([P, KSUB, sz], FP8, tag=f"xl{i % 4}",
                              name=f"xl{i}")
                xhs.append(th)
                xls.append(tl)

            # DMA issue order tuned so the first projection group's operands
            # (wq8, wql, bq, chunk 0) arrive first.
            def dma_in(i):
                nc.sync.dma_start(xhs[i][:], xh_ds[i])
                nc.sync.dma_start(xls[i][:], xl_ds[i])

            for wn in ("wq8", "wql"):
                nc.sync.dma_start(w_sb[wn][:], w_ds[wn])
            nc.sync.dma_start(bq_sb[:], bq_d)
            dma_in(0)
            for wn in ("wk8", "wkl"):
                nc.sync.dma_start(w_sb[wn][:], w_ds[wn])
            nc.sync.dma_start(bk_sb[:], bk_d)
            dma_in(1)
            for wn in ("wvh", "wvl"):
                nc.sync.dma_start(w_sb[wn][:], w_ds[wn])
            dma_in(2)
            nc.sync.dma_start(mkd_sb[:], mkd_d)
            nc.sync.dma_start(mkt_sb[:], mkt_d)
            for i in range(3, len(CHUNKS)):
                dma_in(i)

            def proj_qk(ci, lg, w8, wl, bias, dst):
                """One 256-token DoubleRow group for Q^T or K^T."""
                t0, sz = CHUNKS[ci]
                g0 = lg * G
                pj = pj_ps.tile([P, 2 * G], F32, tag="pj", name="pj")
                sl = pj[:, 0:G]
                terms = ((w8, xhs[ci]), (wl, xhs[ci]))
                if QK_TERMS == 3:
                    terms += ((w8, xls[ci]),)
                nmm = 4 * len(terms)
                n = 0
                for wt, xt in terms:
                    for kp in range(KSUB // 2):
                        nc.tensor.matmul(
                            sl, lhsT=wt[:, 2 * kp:2 * kp + 2, :],
                            rhs=xt[:, 2 * kp:2 * kp + 2, g0:g0 + G],
                            start=(n == 0), stop=(n == nmm - 1),
                            perf_mode=DR, skip_group_check=True)
                        n += 1
                nc.vector.tensor_scalar(
                    dst[:, t0 + g0:t0 + g0 + G], sl,
                    1.0 / WS, bias[:, 0:1],
                    op0=mybir.AluOpType.mult, op1=mybir.AluOpType.add)

            def proj_v(ci):
                """V for one chunk: one PSUM bank, 128-token lb blocks."""
                t0, sz = CHUNKS[ci]
                nlb = sz // P
                pv = pj_ps.tile([P, 4, P], F32, tag="pj", name="pv")
                terms = ((w_sb["wvh"], xhs[ci]), (w_sb["wvl"], xhs[ci]),
                         (w_sb["wvh"], xls[ci]))
                for lo in range(nlb):
                    n = 0
                    for wt, xt in terms:
                        for kp in range(KSUB // 2):
                            nc.tensor.matmul(
                                pv[:, lo, :],
                                lhsT=xt[:, 2 * kp:2 * kp + 2,
                                        lo * P:(lo + 1) * P],
                                rhs=wt[:, 2 * kp:2 * kp + 2, :],
                                start=(n == 0), stop=(n == 11),
                                perf_mode=DR, skip_group_check=True)
                            n += 1
                lb0 = t0 // P
                for h in range(2):
                    nc.vector.tensor_scalar_mul(
                        v_sb[:, h, lb0:lb0 + nlb, 0:DH],
                        pv[:, 0:nlb, h * DH:(h + 1) * DH], 1.0 / WS)

            o_tiles = {}
            o_done = {}

            def flush_super(b, s):
                """Copy a finished 3-qb PSUM super tile to SBUF + DMA out."""
                nslots = min(3, NKB - 3 * s)
                ot = o_tiles.pop((b, s))
                st = ostp.tile([P, 3 * OC], BF16, tag="ost",
                               name=f"ost_{b}_{s}")
                w = nslots * OC
                nc.vector.tensor_copy(
                    st[:, 0:w], ot[:, 0:nslots, :].rearrange(
                        "p s c -> p (s c)"))
                nc.sync.dma_start(out_d[b, s, :, 0:w], st[:, 0:w])

            def pv_block(b, kb, h, pt_ap):
                for qb in range(kb, min(kb + 3, NKB)):
                    s, slot = divmod(qb, 3)
                    qoff = (qb - kb) * P
                    first = (kb == max(qb - 2, 0))
                    if first and slot == 0 and h == 0:
                        o_tiles[(b, s)] = o_ps.tile(
                            [P, 3, OC], F32, tag="o", name=f"o_{b}_{s}")
                    ot = o_tiles[(b, s)]
                    nc.tensor.matmul(
                        ot[:, slot, h * (DH + 1):(h + 1) * (DH + 1)],
                        lhsT=pt_ap[:, qoff:qoff + P],
                        rhs=v_sb[:, h, b * NKB + kb, :],
                        start=(first and slot == 0 and h == 0),
                        stop=(qb == kb), skip_group_check=True)
                    if qb == kb and h == 1:
                        done = o_done.get((b, s), 0) + 1
                        o_done[(b, s)] = done
                        if done == min(3, NKB - 3 * s):
                            flush_super(b, s)

            def attend(b, kb, mask_engine, split=False):
                t0 = b * L
                k0 = t0 + kb * P
                qw = min(QW, L - kb * P)
                st = st_ps.tile([P, 2, 512], F32, tag="st", name="st")
                eng = nc.gpsimd if mask_engine == "pool" else nc.vector
                if split:
                    # per-head exp/mask/PV: shorter critical path for the
                    # tail blocks that run with no projection work left
                    for h in range(2):
                        hs = h * DH
                        nc.tensor.matmul(st[:, h, 0:qw],
                                         lhsT=kt_sb[hs:hs + DH, k0:k0 + P],
                                         rhs=qt_sb[hs:hs + DH, k0:k0 + qw],
                                         start=True, stop=True)
                        pt = ptp.tile([P, QW], BF16, tag=f"pth{h}",
                                      name=f"pt{h}")
                        nc.scalar.activation(
                            pt[:, 0:qw], st[:, h, 0:qw],
                            mybir.ActivationFunctionType.Exp, scale=0.125)
                        eng.tensor_mul(pt[:, 0:P], pt[:, 0:P],
                                       mkd_sb[:, 0, :])
                        if qw == QW:
                            eng.tensor_mul(pt[:, 2 * P:3 * P],
                                           pt[:, 2 * P:3 * P],
                                           mkt_sb[:, 0, :])
                        pv_block(b, kb, h, pt)
                    return
                for h in range(2):
                    hs = h * DH
                    nc.tensor.matmul(st[:, h, 0:qw],
                                     lhsT=kt_sb[hs:hs + DH, k0:k0 + P],
                                     rhs=qt_sb[hs:hs + DH, k0:k0 + qw],
                                     start=True, stop=True)
                pt = ptp.tile([P, 2, QW], BF16, tag="pt", name="pt")
                nc.scalar.activation(
                    pt[:, :, 0:qw], st[:, :, 0:qw],
                    mybir.ActivationFunctionType.Exp, scale=0.125)
                eng.tensor_mul(pt[:, :, 0:P], pt[:, :, 0:P], mkd_sb[:])
                if qw == QW:
                    eng.tensor_mul(pt[:, :, 2 * P:3 * P],
                                   pt[:, :, 2 * P:3 * P], mkt_sb[:])
                for h in range(2):
                    pv_block(b, kb, h, pt[:, h, :])

            # Attend(b, kb) is ready once Q^T/K^T cover batch-local token
            # (kb+3)*128 and V covers key block kb.  Emit attends at the
            # earliest such point so PE always has matmul work while ACT
            # runs exp.
            att_i = [0]

            def pop_ready(b, pend, q_cover, v_cover, tail=False):
                while pend:
                    kb = pend[0]
                    if (min(kb + 3, NKB) * P > q_cover
                            or (kb + 1) * P > v_cover):
                        break
                    pend.pop(0)
                    i = att_i[0]
                    att_i[0] += 1
                    eng = "pool" if 4 <= i < 16 else "dve"
                    attend(b, kb, eng, split=tail and len(pend) < 3)

            for b in range(B):
                pend = list(range(NKB))
                qc, vc = 0, 0
                bchunks = [i for i, (t0, sz) in enumerate(CHUNKS)
                           if t0 // L == b]
                last = bchunks[-1]
                for ci in bchunks:
                    t0, sz = CHUNKS[ci]
                    for lg in range(sz // G):
                        proj_qk(ci, lg, w_sb["wq8"], w_sb["wql"], bq_sb,
                                qt_sb)
                        proj_qk(ci, lg, w_sb["wk8"], w_sb["wkl"], bk_sb,
                                kt_sb)
                        qc = t0 - b * L + (lg + 1) * G
                        pop_ready(b, pend, qc, vc)
                    proj_v(ci)
                    vc = t0 - b * L + sz
                    pop_ready(b, pend, qc, vc, tail=(ci == last))
                assert not pend, pend
    nc.finalize()
    return nc


_NC = None


def _get_nc():
    global _NC
    if _NC is None:
        _NC = build_program()
    return _NC


def _masks():
    pk = np.arange(P)[:, None]
    f = np.arange(P)[None, :]
    mkd = (f >= pk).astype(np.float32)       # diag block: query >= key
    mkt = (f < pk).astype(np.float32)        # tail block: dist <= 255
    mkd2 = np.repeat(mkd[:, None, :], 2, axis=1).astype(ml_dtypes.bfloat16)
    mkt2 = np.repeat(mkt[:, None, :], 2, axis=1).astype(ml_dtypes.bfloat16)
    return np.ascontiguousarray(mkd2), np.ascontiguousarray(mkt2)


def _prepare_in_maps(inputs):
    hs = np.asarray(inputs["hidden_states"], np.float32)
    Wq = np.asarray(inputs["Wq"], np.float32)
    Wk = np.asarray(inputs["Wk"], np.float32)
    Wv = np.asarray(inputs["Wv"], np.float32)
    bq = np.asarray(inputs["bq"], np.float32)
    bk = np.asarray(inputs["bk"], np.float32)

    x_flat = hs.reshape(NT, D)
    # xt[p, k, t] = x_flat[t, k*128+p]
    xt = np.ascontiguousarray(
        x_flat.T.reshape(KSUB, P, NT).transpose(1, 0, 2))
    xh = xt.astype(ml_dtypes.float8_e4m3)
    xl = (xt - xh.astype(np.float32)).astype(ml_dtypes.float8_e4m3)
    chunks = {}
    for i, (t0, sz) in enumerate(CHUNKS):
        chunks[f"xh{i}"] = np.ascontiguousarray(xh[:, :, t0:t0 + sz])
        chunks[f"xl{i}"] = np.ascontiguousarray(xl[:, :, t0:t0 + sz])
    mkd, mkt = _masks()

    def wsplit(W, c):
        # [P, KSUB, 128]: w[p, k, m] = WS * W[k*128+p, c*128+m]
        ws = np.ascontiguousarray(
            (WS * W[:, c * P:(c + 1) * P]).reshape(KSUB, P, P)
            .transpose(1, 0, 2))
        w8 = ws.astype(ml_dtypes.float8_e4m3)
        wl = (ws - w8.astype(np.float32)).astype(ml_dtypes.float8_e4m3)
        return w8, wl

    in_maps = []
    for c in range(NCORES):
        wq8, wql = wsplit(Wq, c)
        wk8, wkl = wsplit(Wk, c)
        wvh, wvl = wsplit(Wv, c)
        m = dict(chunks)
        m.update({
            "wq8": wq8, "wql": wql, "wk8": wk8, "wkl": wkl,
            "wvh": wvh, "wvl": wvl,
            "bq": np.ascontiguousarray(bq[c * P:(c + 1) * P].reshape(P, 1)),
            "bk": np.ascontiguousarray(bk[c * P:(c + 1) * P].reshape(P, 1)),
            "mkd": mkd, "mkt": mkt,
        })
        in_maps.append(m)
    return in_maps


def run(inputs, trace=False, **kwargs):
    nc = _get_nc()
    in_maps = _prepare_in_maps(inputs)
    res = run_bass_kernel_spmd(nc, in_maps, core_ids=list(range(NCORES)),
                               trace=trace, **kwargs)
    bv = np.asarray(inputs["bv"], np.float32)
    outs = []
    for c in range(NCORES):
        o = np.asarray(res.results[c]["out"]).astype(np.float32)
        # [B, NSUP, P, 3, OC]; (s, slot) -> query block 3s+slot, row p
        o = o.reshape(B, NSUP, P, 3, OC).transpose(0, 1, 3, 2, 4)
        o = o.reshape(B, NSUP * 3 * P, OC)[:, :L]      # [B, L, OC]
        for h in range(2):
            c0 = h * (DH + 1)
            outs.append(o[:, :, c0:c0 + DH] / o[:, :, c0 + DH:c0 + DH + 1])
    full = np.concatenate(outs, axis=2)
    full = full + bv[None, None, :]
    return full.astype(np.float32), res


def kernel(**inputs):
    out, _ = run(inputs, trace=False)
    return out
